# revision 4
# baseline (speedup 1.0000x reference)
"""Trainium2 Bass kernel for nn_ComplexNN (3-layer MLP, blended tanh act).

  h1 = blend_act(x @ W1 + b1);  blend_act(z) = z>0 ? 0.9z+0.1tanh(z) : 0.5tanh(z)
  h2 = relu(h1 @ W2 + b2)
  out = h2 @ W3 + b3

Data-parallel over 8 NeuronCores: each core takes 4096 rows of x, weights
replicated. Fully fused on-chip; matmuls in bf16 with fp32 PSUM accumulate.

Layout: activations are kept feature-on-partitions (h1^T, h2^T) so each
matmul's contraction dim lands on partitions with no intermediate
transposes. Only x itself needs a transpose: fp32->bf16 cast via SWDGE DMA
(DRAM->DRAM) then DMA xbar transpose (DRAM->SBUF, 2-byte path).

blend_act decomposition (t = tanh(z)):
  blend(z) = 0.9*relu(z) + 0.1*t + 0.4*min(t, 0)
ACT: t = Tanh(psum + b1), a = Relu(0.9*psum + 0.9*b1)
DVE: m = (t min 0) * 0.4 ; u = (t * 0.1) + a ; h1 = u + m
"""

import sys

sys.path.insert(0, "/opt/trn_rl_repo")

import ml_dtypes
import numpy as np

import concourse.bass as bass
import concourse.mybir as mybir
import concourse.tile as tile
from concourse import bacc
from concourse.bass_utils import run_bass_kernel_spmd

N_CORES = 8
B, D, H, H2, C = 32768, 512, 1024, 512, 10
BL = B // N_CORES  # rows per core = 4096
NCHUNK = 4
NB = BL // NCHUNK  # batch-chunk width = 1024
KD = D // 128      # 4  k-tiles for mm1
KH = H // 128      # 8  k-tiles for mm2 / h-tiles of h1
KH2 = H2 // 128    # 4  k-tiles for mm3 / h2-tiles of h2
NBS = NB // 512    # 512-wide moving-operand slices per chunk
NBT = NB // 128    # 128-wide b-tiles per chunk (mm3)

F32 = mybir.dt.float32
BF16 = mybir.dt.bfloat16
AF = mybir.ActivationFunctionType
ALU = mybir.AluOpType


def _body(ctx, tc, outs, ins):
    nc = tc.nc
    x, w1, w2, w3, b1c, b1s, b2c, b3b = ins
    (out,) = outs

    wpool = ctx.enter_context(tc.tile_pool(name="weights", bufs=1))
    xpool = ctx.enter_context(tc.tile_pool(name="xT", bufs=2 * KD))
    h1pool = ctx.enter_context(tc.tile_pool(name="h1T", bufs=2 * KH))
    h2pool = ctx.enter_context(tc.tile_pool(name="h2T", bufs=2 * KH2))
    tpool = ctx.enter_context(tc.tile_pool(name="tmp", bufs=2))
    opool = ctx.enter_context(tc.tile_pool(name="ostage", bufs=2))
    mmpool = ctx.enter_context(tc.tile_pool(name="mm", bufs=3, space="PSUM"))
    mm3pool = ctx.enter_context(tc.tile_pool(name="mm3", bufs=2, space="PSUM"))
    xbd = ctx.enter_context(tc.tile_pool(name="xbd", bufs=2, space="DRAM"))

    # resident weights / biases
    w1s = wpool.tile([128, KD * H], BF16)     # w1s[p, k*H + h]  = W1[k*128+p, h]
    w2s = wpool.tile([128, KH * H2], BF16)    # w2s[p, k*H2 + m] = W2[k*128+p, m]
    w3s = wpool.tile([128, KH2 * C], BF16)    # w3s[p, k*C + c]  = W3[k*128+p, c]
    b1cs = wpool.tile([128, KH], F32)         # b1cs[p, i] = b1[i*128+p]
    b1ss = wpool.tile([128, KH], F32)         # 0.9 * b1
    b2cs = wpool.tile([128, KH2], F32)
    b3bs = wpool.tile([128, C], F32)          # b3 broadcast to all partitions
    nc.sync.dma_start(out=w1s[:], in_=w1[:])
    nc.sync.dma_start(out=w2s[:], in_=w2[:])
    nc.sync.dma_start(out=w3s[:], in_=w3[:])
    nc.sync.dma_start(out=b1cs[:], in_=b1c[:])
    nc.sync.dma_start(out=b1ss[:], in_=b1s[:])
    nc.sync.dma_start(out=b2cs[:], in_=b2c[:])
    nc.sync.dma_start(out=b3bs[:], in_=b3b[:])

    # out viewed per (chunk, b-tile): row = c*NB + j*128 + p
    out_v = out.rearrange("(c j p) col -> c p j col", c=NCHUNK, j=NBT, p=128)

    for c in range(NCHUNK):
        rows = slice(c * NB, (c + 1) * NB)

        # fp32 -> bf16 cast on the fly (SWDGE), DRAM -> DRAM
        xb = xbd.tile([NB, D], BF16, tag="xb")
        nc.gpsimd.dma_start(out=xb[:], in_=x[rows, :])

        # xbar transpose: xT[k] [128 d, NB b] <- xb[:, k*128:(k+1)*128]
        xT = []
        for k in range(KD):
            xt = xpool.tile([128, NB], BF16, tag="xt")
            nc.sync.dma_start(out=xt[:], in_=xb[:, k * 128 : (k + 1) * 128], transpose=True)
            xT.append(xt)

        # ---- mm1 + blend_act ----  h1T[i] [128 h, NB b]
        h1T = []
        for i in range(KH):
            ps = mmpool.tile([128, NB], F32, tag="ps")
            for bs in range(NBS):
                cols = slice(bs * 512, (bs + 1) * 512)
                for k in range(KD):
                    nc.tensor.matmul(
                        ps[:, cols],
                        w1s[:, k * H + i * 128 : k * H + (i + 1) * 128],
                        xT[k][:, cols],
                        start=(k == 0),
                        stop=(k == KD - 1),
                    )
            t = tpool.tile([128, NB], BF16, tag="t")
            a = tpool.tile([128, NB], BF16, tag="a")
            nc.scalar.activation(t[:], ps[:], AF.Tanh, bias=b1cs[:, i : i + 1], scale=1.0)
            nc.scalar.activation(a[:], ps[:], AF.Relu, bias=b1ss[:, i : i + 1], scale=0.9)
            m = tpool.tile([128, NB], BF16, tag="m")
            u = tpool.tile([128, NB], BF16, tag="u")
            nc.vector.tensor_scalar(m[:], t[:], 0.0, 0.4, ALU.min, ALU.mult)
            nc.vector.scalar_tensor_tensor(u[:], t[:], 0.1, a[:], ALU.mult, ALU.add)
            h1 = h1pool.tile([128, NB], BF16, tag="h1")
            nc.vector.tensor_add(h1[:], u[:], m[:])
            h1T.append(h1)

        # ---- mm2 + relu ----  h2T[j] [128 h2, NB b]
        h2T = []
        for j in range(KH2):
            ps2 = mmpool.tile([128, NB], F32, tag="ps")
            for bs in range(NBS):
                cols = slice(bs * 512, (bs + 1) * 512)
                for k in range(KH):
                    nc.tensor.matmul(
                        ps2[:, cols],
                        w2s[:, k * H2 + j * 128 : k * H2 + (j + 1) * 128],
                        h1T[k][:, cols],
                        start=(k == 0),
                        stop=(k == KH - 1),
                    )
            h2 = h2pool.tile([128, NB], BF16, tag="h2")
            nc.scalar.activation(h2[:], ps2[:], AF.Relu, bias=b2cs[:, j : j + 1], scale=1.0)
            h2T.append(h2)

        # ---- mm3 + bias ----  out[b, c] with b on partitions (h2T as stationary)
        stage = opool.tile([128, NBT * C], F32, tag="stage")
        for bt in range(NBT):
            ps3 = mm3pool.tile([128, C], F32, tag="ps3")
            for k in range(KH2):
                nc.tensor.matmul(
                    ps3[:],
                    h2T[k][:, bt * 128 : (bt + 1) * 128],
                    w3s[:, k * C : (k + 1) * C],
                    start=(k == 0),
                    stop=(k == KH2 - 1),
                )
            nc.vector.tensor_add(stage[:, bt * C : (bt + 1) * C], ps3[:], b3bs[:])
        nc.sync.dma_start(out=out_v[c], in_=stage[:])


_CACHED = None


def _build():
    global _CACHED
    if _CACHED is not None:
        return _CACHED
    nc = bacc.Bacc(
        "TRN2",
        target_bir_lowering=False,
        debug=False,
        enable_asserts=False,
        num_devices=N_CORES,
    )
    x = nc.dram_tensor("x", [BL, D], F32, kind="ExternalInput").ap()
    w1 = nc.dram_tensor("w1", [128, KD * H], BF16, kind="ExternalInput").ap()
    w2 = nc.dram_tensor("w2", [128, KH * H2], BF16, kind="ExternalInput").ap()
    w3 = nc.dram_tensor("w3", [128, KH2 * C], BF16, kind="ExternalInput").ap()
    b1c = nc.dram_tensor("b1c", [128, KH], F32, kind="ExternalInput").ap()
    b1s = nc.dram_tensor("b1s", [128, KH], F32, kind="ExternalInput").ap()
    b2c = nc.dram_tensor("b2c", [128, KH2], F32, kind="ExternalInput").ap()
    b3b = nc.dram_tensor("b3b", [128, C], F32, kind="ExternalInput").ap()
    out = nc.dram_tensor("out", [BL, C], F32, kind="ExternalOutput").ap()

    from contextlib import ExitStack

    with tile.TileContext(nc) as tc, ExitStack() as ctx:
        _body(ctx, tc, [out], [x, w1, w2, w3, b1c, b1s, b2c, b3b])
    nc.compile()
    _CACHED = nc
    return nc


def _prep_weights(W1, b1, W2, b2, W3, b3):
    bf = ml_dtypes.bfloat16
    w1h = np.ascontiguousarray(
        W1.astype(bf).reshape(KD, 128, H).transpose(1, 0, 2).reshape(128, KD * H)
    )
    w2h = np.ascontiguousarray(
        W2.astype(bf).reshape(KH, 128, H2).transpose(1, 0, 2).reshape(128, KH * H2)
    )
    w3h = np.ascontiguousarray(
        W3.astype(bf).reshape(KH2, 128, C).transpose(1, 0, 2).reshape(128, KH2 * C)
    )
    b1f = b1.astype(np.float32)
    b1ch = np.ascontiguousarray(b1f.reshape(KH, 128).T)
    b1sh = np.ascontiguousarray((0.9 * b1f).reshape(KH, 128).T)
    b2ch = np.ascontiguousarray(b2.astype(np.float32).reshape(KH2, 128).T)
    b3bh = np.ascontiguousarray(np.tile(b3.astype(np.float32), (128, 1)))
    return w1h, w2h, w3h, b1ch, b1sh, b2ch, b3bh


def kernel(x, W1, b1, W2, b2, W3, b3):
    x = np.asarray(x, dtype=np.float32)
    nc = _build()
    w1h, w2h, w3h, b1ch, b1sh, b2ch, b3bh = _prep_weights(
        np.asarray(W1), np.asarray(b1), np.asarray(W2), np.asarray(b2),
        np.asarray(W3), np.asarray(b3),
    )
    in_maps = []
    for i in range(N_CORES):
        in_maps.append(
            {
                "x": np.ascontiguousarray(x[i * BL : (i + 1) * BL]),
                "w1": w1h, "w2": w2h, "w3": w3h,
                "b1c": b1ch, "b1s": b1sh, "b2c": b2ch, "b3b": b3bh,
            }
        )
    res = run_bass_kernel_spmd(nc, in_maps, core_ids=list(range(N_CORES))).results
    return np.concatenate([r["out"] for r in res], axis=0)
